# revision 53
# baseline (speedup 1.0000x reference)
"""Luong 'concat' attention TRN2 Bass kernel (v2).

Problem: B=64, S=2048, D=512 (enc_dim == dec_dim), fp32.
  hidden = tanh(enc @ W_enc^T + ht @ W_dec^T + W_b)   [B, S, D]
  scores = hidden @ V_w^T (+ V_b)                     [B, 1, S]
  weights = softmax(scores, axis=-1)
  c_t = weights @ enc                                 [B, 1, D]

Sharding: data-parallel over batch, 8 batches per core on 8 cores.

Per-core dataflow (per batch):
  1. Casting DMA (gpsimd/SWDGE): enc f32 HBM -> enc_nat bf16 SBUF.
  2. PE transposes enc_nat (bf16) -> PSUM; DVE copies PSUM -> encT fp8
     ([128 e-part, 4 et, 2048 s]).
  3. mm1 on PE in fp8e4 DoubleRow mode (2 k-tiles of 128 per instr, 0.5
     cyc/row): psum = 32*W_enc@enc.  Two chains: W_hi = q(32*W) and
     W_lo = q(32*(W - W_hi/32)) residual, so effective W precision is
     ~bf16 while running at fp8 speed.
  4. ACT tanh with per-partition bias (ht@W_dec^T + W_b) and scale=1/32,
     merged over 2 s-windows per instr -> hT bf16 [128 d-part, 4 dt, 2048].
  5. scoresT: 64 tiny bf16 matmuls (lhsT=hT chunk, rhs=V column) ->
     psum scT [128 s-part, 16 s-chunks] (pre-zeroed bank, start=False).
  6. One ACT exp over [128,16] -> uT bf16 + accum_out per-partition sums;
     tiny f32r matmul dots those with ones -> total softmax denominator.
  7. c_t = sum_s u[s] * enc[s,:]: 16 bf16 matmuls lhsT=uT[:,j], rhs=
     enc_nat[:,j,:] accumulated in psum [1,512]; DVE scales by 1/den.
  V_b dropped (softmax shift-invariant).
"""

import numpy as np

B, S, D = 64, 2048, 512
N_CORES = 8
BPC = B // N_CORES      # batches per core
ET = D // 128           # 4 e-tiles
DT = D // 128           # 4 d-tiles
SC = S // 512           # 4 s-windows of 512
SCJ = S // 128          # 16 s-chunks of 128
WSCALE = 32.0           # fp8 weight pre-scale (undone by tanh scale=1/32)
WCORR = False           # W fp8 residual-correction chain (off: mm1 is
                        # hi-chain only; rel err ~1.1e-2 vs 4.9e-3, but
                        # ~8us faster and still 1.7x under the 2e-2 gate)

_CACHE = {}


def _build(wcorr=WCORR):
    import concourse.bacc as bacc
    import concourse.tile as tile
    from concourse import mybir
    from concourse.masks import make_identity

    f32 = mybir.dt.float32
    f32r = mybir.dt.float32r
    bf16 = mybir.dt.bfloat16
    fp8 = mybir.dt.float8e4
    Tanh = mybir.ActivationFunctionType.Tanh
    Exp = mybir.ActivationFunctionType.Exp
    Copy = mybir.ActivationFunctionType.Copy
    DR = mybir.MatmulPerfMode.DoubleRow
    AX = mybir.AxisListType.X
    add_op = mybir.AluOpType.add

    nc = bacc.Bacc(None, target_bir_lowering=False, debug=False)
    enc = nc.dram_tensor("enc_outs", [BPC, S, D], f32, kind="ExternalInput").ap()
    ht = nc.dram_tensor("ht", [1, BPC, D], f32, kind="ExternalInput").ap()
    W_w = nc.dram_tensor("W_w", [D, 2 * D], f32, kind="ExternalInput").ap()
    W_b = nc.dram_tensor("W_b", [D], f32, kind="ExternalInput").ap()
    V_w = nc.dram_tensor("V_w", [1, D], f32, kind="ExternalInput").ap()
    out = nc.dram_tensor("c_t", [BPC, 1, D], f32, kind="ExternalOutput").ap()

    with tile.TileContext(nc) as tc:
        with tc.tile_pool(name="const", bufs=1) as const, \
             tc.tile_pool(name="enc_nat", bufs=4) as enc_pool, \
             tc.tile_pool(name="encT", bufs=3) as encT_pool, \
             tc.tile_pool(name="hT", bufs=3) as hT_pool, \
             tc.tile_pool(name="uT", bufs=2) as uT_pool, \
             tc.tile_pool(name="small", bufs=8) as small_pool, \
             tc.tile_pool(name="pp_mm", bufs=2, space="PSUM") as pp_mm, \
             tc.tile_pool(name="pp_t", bufs=2, space="PSUM") as pp_t, \
             tc.tile_pool(name="pp_sc", bufs=1, space="PSUM") as pp_sc, \
             tc.tile_pool(name="pp_ct", bufs=1, space="PSUM") as pp_ct:

            ident = const.tile([128, 128], f32)
            make_identity(nc, ident)
            ident_bf = const.tile([128, 128], bf16)
            nc.scalar.copy(out=ident_bf, in_=ident)

            ones_f = const.tile([128, 1], f32)
            nc.vector.memset(ones_f, 1.0)

            # ---- weights: casting DMA to bf16 natural, transpose on PE.
            # Encoder half first so mm1's weights are ready ASAP; the
            # decoder half (bias only) follows behind batch-0's enc load.
            wnat = const.tile([128, DT, 2 * D], bf16)
            nc.gpsimd.dma_start(
                out=wnat[:, :, 0:D],
                in_=W_w[:, 0:D].rearrange("(dc p) e -> p dc e", p=128),
            )
            nc.gpsimd.dma_start(
                out=wnat[:, :, D:2 * D],
                in_=W_w[:, D:2 * D].rearrange("(dc p) e -> p dc e", p=128),
            )
            w8hi = const.tile([128, ET, D], fp8, name="w8hi")
            w8lo = const.tile([128, ET, D], fp8, name="w8lo") if wcorr else None
            wTf = const.tile([128, ET, D], f32, name="wTf")
            w_decT = const.tile([128, ET, D], f32r, name="w_decT")
            for et in range(ET):
                # encoder half: W_w[:, et*128 : (et+1)*128] -> [e-part, d]
                pw = pp_t.tile([128, 2, 512], bf16, tag="pt")
                pwf = pw.rearrange("p a b -> p (a b)")
                for dc in range(DT):
                    nc.tensor.transpose(
                        out=pwf[:, dc * 128:(dc + 1) * 128],
                        in_=wnat[:, dc, et * 128:(et + 1) * 128],
                        identity=ident_bf,
                    )
                nc.vector.tensor_copy(out=wTf[:, et, :], in_=pwf[:, 0:D])
                nc.scalar.mul(out=w8hi[:, et, :], in_=pwf[:, 0:D], mul=WSCALE)
                # decoder half: W_w[:, D + et*128 : ...]
                pw2 = pp_t.tile([128, 2, 512], bf16, tag="pt")
                pw2f = pw2.rearrange("p a b -> p (a b)")
                for dc in range(DT):
                    nc.tensor.transpose(
                        out=pw2f[:, dc * 128:(dc + 1) * 128],
                        in_=wnat[:, dc, D + et * 128:D + (et + 1) * 128],
                        identity=ident_bf,
                    )
                nc.scalar.copy(out=w_decT[:, et, :], in_=pw2f[:, 0:D])
            if wcorr:
                mult = mybir.AluOpType.mult
                sub = mybir.AluOpType.subtract
                for et in range(ET):
                    # w8lo = q(32*W - w8hi_vals) = q(32*(W - q(32W)/32))
                    nc.vector.scalar_tensor_tensor(
                        out=w8lo[:, et, :], in0=wTf[:, et, :], scalar=WSCALE,
                        in1=w8hi[:, et, :], op0=mult, op1=sub,
                    )

            # ---- W_b -> per-partition cols wb_pcol[128, dt] ----
            wbrow = const.tile([1, D], f32)
            nc.sync.dma_start(out=wbrow, in_=W_b.rearrange("(o d) -> o d", o=1))
            wb_pcol = const.tile([128, DT], f32)
            for dc in range(DT):
                pv = pp_sc.tile([128, 32], f32, tag="sc")
                nc.tensor.transpose(
                    out=pv[:, 0:1],
                    in_=wbrow[0:1, dc * 128:(dc + 1) * 128],
                    identity=ident[0:1, 0:1],
                )
                nc.scalar.copy(out=wb_pcol[:, dc:dc + 1], in_=pv[:, 0:1])

            # ---- V -> v_col bf16 [128 d-part, dt] ----
            vrow = const.tile([1, D], f32)
            nc.sync.dma_start(out=vrow, in_=V_w)
            v_col = const.tile([128, DT], bf16)
            for dt_i in range(DT):
                pv = pp_sc.tile([128, 32], f32, tag="sc")
                nc.tensor.transpose(
                    out=pv[:, 0:1],
                    in_=vrow[0:1, dt_i * 128:(dt_i + 1) * 128],
                    identity=ident[0:1, 0:1],
                )
                nc.scalar.copy(out=v_col[:, dt_i:dt_i + 1], in_=pv[:, 0:1])

            # ---- ht -> htT f32r [128 e-part, et, b]; bias_db[128, dt, b] ----
            htn = const.tile([BPC, D], f32)
            nc.sync.dma_start(out=htn, in_=ht[0])
            htT = const.tile([128, ET, BPC], f32r)
            for ec in range(ET):
                pv = pp_sc.tile([128, 32], f32, tag="sc")
                nc.tensor.transpose(
                    out=pv[:, 0:BPC],
                    in_=htn[:, ec * 128:(ec + 1) * 128],
                    identity=ident[0:BPC, 0:BPC],
                )
                nc.scalar.copy(out=htT[:, ec, :], in_=pv[:, 0:BPC])
            bias_db = const.tile([128, DT, BPC], f32, name="bias_db")
            for dt_i in range(DT):
                pb = pp_sc.tile([128, 32], f32, tag="sc")
                for ec in range(ET):
                    nc.tensor.matmul(
                        out=pb[:, 0:BPC],
                        lhsT=w_decT[:, ec, dt_i * 128:(dt_i + 1) * 128],
                        rhs=htT[:, ec, :],
                        start=(ec == 0), stop=(ec == ET - 1),
                    )
                nc.vector.tensor_scalar_add(
                    out=bias_db[:, dt_i, :], in0=pb[:, 0:BPC],
                    scalar1=wb_pcol[:, dt_i:dt_i + 1],
                )

            # ================= main loop =================
            # Software-pipelined: batch b's softmax/c_t phase is emitted
            # between batch b+1's transposes and mm1, so the in-order PE
            # queue never stalls waiting for ACT's exp.
            prev = None  # state dict for the software pipeline

            def phase2a(st):
                # scores for s-chunks 0..7 (h0 half), exp0, c_t j 0..7
                b, en, hT = st["b"], st["en"], st["hT"]
                # no memset: the first matmul's start=True marks the whole
                # psum zero-region pending, so each column's first
                # accumulation auto-assigns over stale data
                scT = pp_sc.tile([128, 32], f32, tag="sc")
                for j in range(SCJ // 2):
                    for dt_i in range(DT):
                        nc.tensor.matmul(
                            out=scT[:, j:j + 1],
                            lhsT=hT[:, dt_i, j * 128:(j + 1) * 128],
                            rhs=v_col[:, dt_i:dt_i + 1],
                            start=(j == 0 and dt_i == 0),
                            stop=(dt_i == DT - 1),
                            skip_group_check=True,
                        )
                uT = uT_pool.tile([128, SCJ], bf16, tag="uT")
                denp = small_pool.tile([128, 2], f32, tag="denp")
                nc.scalar.activation(
                    out=uT[:, 0:8], in_=scT[:, 0:8], func=Exp,
                    accum_out=denp[:, 0:1],
                )
                ct_ps = pp_ct.tile([1, D], f32, tag="ct")
                for j in range(SCJ // 2):
                    nc.tensor.matmul(
                        out=ct_ps, lhsT=uT[:, j:j + 1], rhs=en[:, j, :],
                        start=(j == 0), stop=False,
                    )
                st.update(scT=scT, uT=uT, denp=denp, ct_ps=ct_ps)

            def phase2b(st):
                b, en, hT = st["b"], st["en"], st["hT"]
                scT, uT, denp, ct_ps = (st["scT"], st["uT"], st["denp"],
                                        st["ct_ps"])
                for j in range(SCJ // 2, SCJ):
                    for dt_i in range(DT):
                        nc.tensor.matmul(
                            out=scT[:, j:j + 1],
                            lhsT=hT[:, dt_i, j * 128:(j + 1) * 128],
                            rhs=v_col[:, dt_i:dt_i + 1],
                            start=False, stop=(dt_i == DT - 1),
                            skip_group_check=True,
                        )
                nc.scalar.activation(
                    out=uT[:, 8:16], in_=scT[:, 8:16], func=Exp,
                    accum_out=denp[:, 1:2],
                )
                denp_s = small_pool.tile([128, 1], f32, tag="denp_s")
                nc.vector.tensor_reduce(
                    out=denp_s, in_=denp, axis=AX, op=add_op,
                )
                nc.tensor.matmul(
                    out=scT[0:1, 16:17],
                    lhsT=denp_s, rhs=ones_f,
                    start=False, stop=True, skip_group_check=True,
                )
                rden = small_pool.tile([1, 1], f32, tag="rden")
                nc.vector.reciprocal(out=rden, in_=scT[0:1, 16:17])
                for j in range(SCJ // 2, SCJ):
                    nc.tensor.matmul(
                        out=ct_ps, lhsT=uT[:, j:j + 1], rhs=en[:, j, :],
                        start=False, stop=(j == SCJ - 1),
                    )
                ct_sb = small_pool.tile([1, D], f32, tag="ct_sb")
                nc.scalar.activation(
                    out=ct_sb, in_=ct_ps, func=Copy, scale=rden,
                )
                nc.sync.dma_start(out=out[b], in_=ct_sb)

            for b in range(BPC):
                # 1. casting DMA: enc f32 -> bf16 natural [128, scj, 512]
                en = enc_pool.tile([128, SCJ, D], bf16, tag="en",
                                   name=f"en_b{b}")
                for sc in range(SC):
                    nc.gpsimd.dma_start(
                        out=en[:, sc * 4:(sc + 1) * 4, :],
                        in_=enc[b, sc * 512:(sc + 1) * 512, :].rearrange(
                            "(scj p) e -> p scj e", p=128
                        ),
                    )

                # 2. transpose to encT fp8 [128 e-part, et, s].  The
                # previous batch's softmax/c_t matmuls are woven between
                # transpose windows: PE fills two PSUM banks in ~430ns but
                # DVE drains one copy in ~1.2us, so without filler work PE
                # would stall on every bank hand-off.
                encT8 = encT_pool.tile([128, ET, S], fp8, tag="encT",
                                       name=f"encT_b{b}")

                def transp_window(sc):
                    for etp in range(2):          # et-pairs per psum bank
                        pt = pp_t.tile([128, 2, 512], bf16, tag="pt")
                        for et2 in range(2):
                            for scj in range(4):
                                nc.tensor.transpose(
                                    out=pt[:, et2, scj * 128:(scj + 1) * 128],
                                    in_=en[:, sc * 4 + scj,
                                           (etp * 2 + et2) * 128:
                                           (etp * 2 + et2 + 1) * 128],
                                    identity=ident_bf,
                                )
                        # all copies on DVE: ACT must stay clear for tanh,
                        # which gates mm1's psum-bank recycling
                        nc.vector.tensor_copy(
                            out=encT8[:, etp * 2:etp * 2 + 2,
                                      sc * 512:(sc + 1) * 512],
                            in_=pt,
                        )

                transp_window(0)
                transp_window(1)
                if prev is not None:
                    phase2a(prev)
                transp_window(2)
                if prev is not None:
                    phase2b(prev)
                    prev = None
                transp_window(3)

                # 3+4. mm1 (fp8 DR, hi+lo chains) + fused bias/tanh
                hT = hT_pool.tile([128, DT, S], bf16, tag="hT",
                                  name=f"hT_b{b}")
                for h in range(2):                # halves of the batch's S
                    for dt_i in range(DT):
                        ph = pp_mm.tile([128, 1024], f32, tag="mm")
                        for w in range(2):
                            sc = h * 2 + w
                            chains = [w8hi, w8lo] if wcorr else [w8hi]
                            n_in = 2 * len(chains)
                            i = 0
                            for wmat in chains:
                                for j in range(2):
                                    nc.tensor.matmul(
                                        out=ph[:, w * 512:(w + 1) * 512],
                                        lhsT=wmat[:, 2 * j:2 * j + 2,
                                                  dt_i * 128:(dt_i + 1) * 128],
                                        rhs=encT8[:, 2 * j:2 * j + 2,
                                                  sc * 512:(sc + 1) * 512],
                                        start=(i == 0), stop=(i == n_in - 1),
                                        perf_mode=DR,
                                    )
                                    i += 1
                        nc.scalar.activation(
                            out=hT[:, dt_i, h * 1024:(h + 1) * 1024],
                            in_=ph, func=Tanh,
                            bias=bias_db[:, dt_i, b:b + 1],
                            scale=1.0 / WSCALE,
                        )
                        if h == 1 and b == BPC - 1 and dt_i == 1:
                            # last batch: start its own scores/exp for the
                            # h0 half under the remaining mm1 work
                            prev = {"b": b, "en": en, "hT": hT}
                            phase2a(prev)
                if prev is None or prev["b"] != b:
                    prev = {"b": b, "en": en, "hT": hT}

            phase2b(prev)

    nc.compile()
    return nc


def _get_nc():
    if "nc" not in _CACHE:
        _CACHE["nc"] = _build()
    return _CACHE["nc"]


def _run(inputs, trace=False, **kw):
    from concourse.bass_utils import run_bass_kernel_spmd

    nc = _get_nc()
    enc = np.asarray(inputs["enc_outs"], dtype=np.float32)
    ht = np.asarray(inputs["ht"], dtype=np.float32)
    W_w = np.asarray(inputs["W_w"], dtype=np.float32)
    W_b = np.asarray(inputs["W_b"], dtype=np.float32)
    V_w = np.asarray(inputs["V_w"], dtype=np.float32)
    in_maps = []
    for c in range(N_CORES):
        sl = slice(c * BPC, (c + 1) * BPC)
        in_maps.append({
            "enc_outs": enc[sl],
            "ht": ht[:, sl],
            "W_w": W_w,
            "W_b": W_b,
            "V_w": V_w,
        })
    res = run_bass_kernel_spmd(
        nc, in_maps, core_ids=list(range(N_CORES)), trace=trace, **kw
    )
    full = np.concatenate([res.results[c]["c_t"] for c in range(N_CORES)], axis=0)
    return full, res


def kernel(**inputs) -> np.ndarray:
    out, _ = _run(inputs, trace=False)
    return out
